# revision 7
# baseline (speedup 1.0000x reference)
"""Bass/Tile kernel for nn_CrossAttention_RoPE on TRN2, data-parallel over batch.

v2: 4-stage software pipeline (Qproj+RoPE / logits+softmax / transpose+PV /
output projection) with per-tag PSUM rings, GpSimd offload of elementwise ops,
and per-head tensor_scalar softmax divides that emit the transpose-ready tail
layout directly.
"""
import numpy as np
import concourse.bass as bass
import concourse.mybir as mybir
import concourse.tile as tile
from concourse import bacc
from concourse.bass_utils import run_bass_kernel_spmd
from concourse.masks import make_identity

F32 = mybir.dt.float32
BF16 = mybir.dt.bfloat16

import os
USE_GP_SQ = os.environ.get("K_GP_SQ", "1") == "1"
USE_GP_HAT = os.environ.get("K_GP_HAT", "1") == "1"
USE_GP_QB = os.environ.get("K_GP_QB", "1") == "1"
USE_TS_DAP = os.environ.get("K_TS_DAP", "1") == "1"

# ---- problem constants ----
B, L, C, Lk, H, D = 8, 1704, 1024, 144, 16, 64
LP = 1792           # L padded to 14*128
NLT = LP // 128     # 14 L tiles
GN = 2              # l-tiles per pipeline group
NG = NLT // GN      # 7 groups
MAX_SCALE_MUL = float(np.log(100.0))
HG3 = [(0, 3), (3, 3), (6, 3), (9, 3), (12, 3), (15, 1)]  # logits head tiles
PRT = [(0, 3), (3, 3), (6, 2)]  # tail transpose batches (pairs)


def precompute_freqs_cis(dim, patch_nums, theta=10000.0):
    freqs = 1.0 / theta ** (np.arange(0, dim, 4)[: dim // 4].astype(np.float32) / dim)
    tx, ty = [], []
    grid = 32.0
    for p in patch_nums:
        ix, iy = np.meshgrid(np.arange(p), np.arange(p), indexing="ij")
        tx.append(ix.flatten().astype(np.float32) / p * grid)
        ty.append(iy.flatten().astype(np.float32) / p * grid)
    tx = np.concatenate(tx)
    ty = np.concatenate(ty)
    ang = np.concatenate([np.outer(tx, freqs), np.outer(ty, freqs)], axis=1).astype(np.float32)
    return np.stack([np.cos(ang), np.sin(ang)], axis=-1)  # [Lx, dim//2, 2]


def rope_tables(fc, n_rows):
    """fc: [n, 32, 2] -> C [n_rows, 64] (cos dup), NS [n_rows, 32] (-sin), PS [n_rows, 32] (+sin)."""
    n = fc.shape[0]
    Ct = np.zeros((n_rows, 64), np.float32)
    NS = np.zeros((n_rows, 32), np.float32)
    PS = np.zeros((n_rows, 32), np.float32)
    cos, sin = fc[..., 0], fc[..., 1]
    Ct[:n, 0::2] = cos
    Ct[:n, 1::2] = cos
    NS[:n] = -sin
    PS[:n] = sin
    return Ct, NS, PS


def host_prep(inputs, proj_np=np.float32, with_bias=False):
    """Full inputs -> per-core list of dicts."""
    x = np.asarray(inputs["x"], np.float32)
    y = np.asarray(inputs["y"], np.float32)
    fc = np.asarray(inputs["freqs_cis"], np.float32)
    ab = np.asarray(inputs["attn_bias"], np.float32).reshape(L, Lk)
    Wq = np.asarray(inputs["Wq"], np.float32)
    Wkv = np.asarray(inputs["Wkv"], np.float32)
    Wproj = np.asarray(inputs["Wproj"], np.float32)
    sm = np.exp(np.minimum(np.asarray(inputs["scale_mul"], np.float32), MAX_SCALE_MUL)).reshape(H)

    Cq, NSq, PSq = rope_tables(fc, LP)
    fck = precompute_freqs_cis(D, [12])
    Ck, NSk, PSk = rope_tables(fck, Lk)

    import ml_dtypes
    bias2d = np.zeros((LP, Lk), np.float32)
    bias2d[:L] = ab
    bias3 = np.tile(bias2d, (1, 3)).astype(ml_dtypes.bfloat16)

    shared = {
        "wqT": np.ascontiguousarray(Wq.T).astype(proj_np),
        "wkT": np.ascontiguousarray(Wkv[:C].T).astype(proj_np),
        "wvT": np.ascontiguousarray(Wkv[C:].T).astype(proj_np),
        "wpT": np.ascontiguousarray(Wproj.T).astype(proj_np),
        "qbias": np.asarray(inputs["q_bias"], np.float32),
        "vbias": np.asarray(inputs["v_bias"], np.float32),
        "bproj": np.asarray(inputs["b_proj"], np.float32),
        "smv": sm.astype(np.float32),
        "cq": Cq.astype(ml_dtypes.bfloat16), "nsq": NSq.astype(ml_dtypes.bfloat16),
        "psq": PSq.astype(ml_dtypes.bfloat16),
        "ck": Ck.astype(ml_dtypes.bfloat16), "nsk": NSk.astype(ml_dtypes.bfloat16),
        "psk": PSk.astype(ml_dtypes.bfloat16),
        "bias3": bias3,
    }
    if not with_bias:
        for k in ("qbias", "vbias", "bproj"):
            shared.pop(k)
    xTp = np.zeros((B, C, LP), np.float32)
    xTp[:, :, :L] = x.transpose(0, 2, 1)
    in_maps = []
    for b in range(B):
        m = dict(shared)
        m["xT"] = np.ascontiguousarray(xTp[b]).astype(proj_np)
        m["yT"] = np.ascontiguousarray(y[b].T).astype(proj_np)
        in_maps.append(m)
    return in_maps


def build(dt_proj=BF16, dt_att=BF16, with_bias=False):
    nc = bacc.Bacc("TRN2", target_bir_lowering=False, debug=False, num_devices=8)
    dram = {}
    PROJ_NAMES = {"xT", "yT", "wqT", "wkT", "wvT", "wpT"}
    ATT_NAMES = {"bias3", "cq", "nsq", "psq", "ck", "nsk", "psk"}
    for name, shape in [
        ("xT", [C, LP]), ("yT", [C, Lk]),
        ("wqT", [C, C]), ("wkT", [C, C]), ("wvT", [C, C]), ("wpT", [C, C]),
        ("qbias", [C]), ("vbias", [C]), ("bproj", [C]), ("smv", [H]),
        ("cq", [LP, 64]), ("nsq", [LP, 32]), ("psq", [LP, 32]),
        ("ck", [Lk, 64]), ("nsk", [Lk, 32]), ("psk", [Lk, 32]),
        ("bias3", [LP, 3 * Lk]),
    ]:
        dt = dt_proj if name in PROJ_NAMES else (dt_att if name in ATT_NAMES else F32)
        if name in ("qbias", "vbias", "bproj") and not with_bias:
            continue
        dram[name] = nc.dram_tensor(name, shape, dt, kind="ExternalInput").ap()
    out_d = nc.dram_tensor("out", [LP, C], F32, kind="ExternalOutput").ap()

    with tile.TileContext(nc) as tc:
        kernel_body(tc, dram, out_d, dt_proj, dt_att, with_bias)
    nc.compile()
    return nc


def kernel_body(tc, dram, out_d, dt_proj, dt_att, with_bias):
    nc = tc.nc
    AX = mybir.AxisListType.X
    AF = mybir.ActivationFunctionType
    OP = mybir.AluOpType

    from contextlib import ExitStack
    ctx = ExitStack()
    sb = ctx.enter_context(tc.tile_pool(name="sb", bufs=1))
    ps = ctx.enter_context(tc.tile_pool(name="ps", bufs=1, space="PSUM"))

    def mm(out, lhsT, rhs, start, stop, **kw):
        nc.tensor.matmul(out, lhsT, rhs, start=start, stop=stop, **kw)

    def tr(out, in_, idt, start=True, stop=True):
        nc.tensor.matmul(out, in_, idt, is_transpose=True, start=start, stop=stop,
                         skip_group_check=True,
                         tile_position=(in_.base_partition(), out.base_partition()))

    # ---- constants ----
    ident = sb.tile([128, 128], dt_att, tag="ident")
    make_identity(nc, ident[:])
    eps = sb.tile([128, 1], F32, tag="eps")
    nc.vector.memset(eps[:], 1e-20)
    sm_r = sb.tile([128, H], F32, tag="smr")
    nc.sync.dma_start(sm_r[:], dram["smv"].unsqueeze(0).to_broadcast((128, H)))
    if with_bias:
        qbias_r = sb.tile([128, C], F32, tag="qbr")
        nc.sync.dma_start(qbias_r[:], dram["qbias"].unsqueeze(0).to_broadcast((128, C)))
        vbias_r = sb.tile([128, C], F32, tag="vbr")
        nc.sync.dma_start(vbias_r[:], dram["vbias"].unsqueeze(0).to_broadcast((128, C)))
        bproj_r = sb.tile([128, C], F32, tag="bpr")
        nc.sync.dma_start(bproj_r[:], dram["bproj"].unsqueeze(0).to_broadcast((128, C)))

    def load_w(name, tag):
        ts_ = []
        for kc in range(8):
            t = sb.tile([128, C], dt_proj, tag=tag, bufs=8)
            nc.sync.dma_start(t[:], dram[name][kc * 128:(kc + 1) * 128, :])
            ts_.append(t)
        return ts_

    wk = load_w("wkT", "wk")
    wv = load_w("wvT", "wv")

    yt = []
    for kc in range(8):
        t = sb.tile([128, Lk], dt_proj, tag="yt", bufs=8)
        nc.sync.dma_start(t[:], dram["yT"][kc * 128:(kc + 1) * 128, :])
        yt.append(t)

    # ---- K/V projections: [Lk(128+16), C] ----
    def kv_proj(wtiles, bias_rep, tagp):
        mats = []
        for mt, msz in [(0, 128), (1, 16)]:
            sbt = sb.tile([msz, C], dt_att, tag=f"{tagp}{mt}")
            for nc2 in range(2):
                p = ps.tile([msz, 512], F32, tag="qp", bufs=2)
                for kc in range(8):
                    mm(p[:], yt[kc][:, mt * 128: mt * 128 + msz],
                       wtiles[kc][:, nc2 * 512:(nc2 + 1) * 512],
                       (kc == 0), (kc == 7))
                if bias_rep is None:
                    nc.scalar.copy(sbt[:, nc2 * 512:(nc2 + 1) * 512], p[:])
                else:
                    nc.vector.scalar_tensor_tensor(
                        sbt[:, nc2 * 512:(nc2 + 1) * 512], p[:], 1.0,
                        bias_rep[:msz, nc2 * 512:(nc2 + 1) * 512],
                        op0=OP.mult, op1=OP.add)
            mats.append(sbt)
        return mats

    k_nat = kv_proj(wk, None, "kn")
    wq = load_w("wqT", "wq")

    # ---- rope tables for k (cross freqs) ----
    ckt = sb.tile([128, 64], dt_att, tag="ckt")
    nskt = sb.tile([128, 32], dt_att, tag="nskt")
    pskt = sb.tile([128, 32], dt_att, tag="pskt")
    nc.sync.dma_start(ckt[:], dram["ck"][0:128, :])
    nc.sync.dma_start(nskt[:], dram["nsk"][0:128, :])
    nc.sync.dma_start(pskt[:], dram["psk"][0:128, :])
    ckt2 = sb.tile([16, 64], dt_att, tag="ckt2")
    nskt2 = sb.tile([16, 32], dt_att, tag="nskt2")
    pskt2 = sb.tile([16, 32], dt_att, tag="pskt2")
    nc.sync.dma_start(ckt2[:], dram["ck"][128:Lk, :])
    nc.sync.dma_start(nskt2[:], dram["nsk"][128:Lk, :])
    nc.sync.dma_start(pskt2[:], dram["psk"][128:Lk, :])

    def norm_rope_k(src, msz, ct, nst, pst, scale_rep, tagp):
        """DVE-only norm+rope for the K side (prologue, small)."""
        sq = sb.tile([msz, C], F32, tag=f"{tagp}sq")
        nc.scalar.activation(sq[:], src[:], AF.Square)
        s16 = sb.tile([msz, H], F32, tag=f"{tagp}s16")
        nc.vector.reduce_sum(s16[:], sq[:].rearrange("p (h d) -> p h d", d=D), axis=AX)
        rt = sb.tile([msz, H], F32, tag=f"{tagp}rt")
        nc.scalar.activation(rt[:], s16[:], AF.Sqrt, bias=eps[:msz, :])
        rq = sb.tile([msz, H], F32, tag=f"{tagp}rq")
        nc.vector.reciprocal(rq[:], rt[:])
        if scale_rep is not None:
            nc.vector.tensor_mul(rq[:], rq[:], scale_rep[:msz, :])
        hat = sb.tile([msz, C], dt_att, tag=f"{tagp}hat")
        nc.vector.tensor_mul(
            hat[:].rearrange("p (h d) -> p h d", d=D),
            src[:].rearrange("p (h d) -> p h d", d=D),
            rq[:].unsqueeze(2).to_broadcast((msz, H, D)))
        qa = sb.tile([msz, C], dt_att, tag=f"{tagp}qa")
        nc.vector.tensor_mul(
            qa[:].rearrange("p (h d) -> p h d", d=D),
            hat[:].rearrange("p (h d) -> p h d", d=D),
            ct[:msz, :].unsqueeze(1).to_broadcast((msz, H, D)))
        qb = sb.tile([msz, C], dt_att, tag=f"{tagp}qb")
        hat4 = hat[:].rearrange("p (h j t) -> p h j t", j=32, t=2)
        qb4 = qb[:].rearrange("p (h j t) -> p h j t", j=32, t=2)
        nc.vector.tensor_mul(
            qb4[:, :, :, 0:1].squeeze(3),
            hat4[:, :, :, 1:2].squeeze(3),
            nst[:msz, :].unsqueeze(1).to_broadcast((msz, H, 32)))
        nc.vector.tensor_mul(
            qb4[:, :, :, 1:2].squeeze(3),
            hat4[:, :, :, 0:1].squeeze(3),
            pst[:msz, :].unsqueeze(1).to_broadcast((msz, H, 32)))
        kp = sb.tile([msz, C], dt_att, tag=f"{tagp}kp")
        nc.vector.tensor_add(kp[:], qa[:], qb[:])
        return kp

    # K gets the per-head scale_mul folded in (logits = q_hat . (sm*k_hat))
    kp_m = norm_rope_k(k_nat[0], 128, ckt, nskt, pskt, sm_r, "km")
    kp_t = norm_rope_k(k_nat[1], 16, ckt2, nskt2, pskt2, sm_r, "kt")

    kT = []
    for t in range(8):
        p = ps.tile([128, Lk], dt_att, tag="tr2", bufs=2)
        for hh in range(2):
            h = 2 * t + hh
            tr(p[64 * hh:64 * hh + 64, 0:128], kp_m[:, h * D:(h + 1) * D], ident[:])
            tr(p[64 * hh:64 * hh + 64, 128:Lk], kp_t[:, h * D:(h + 1) * D],
               ident[:16, :16])
        sbt = sb.tile([128, Lk], dt_att, tag="kT", bufs=8)
        nc.vector.tensor_copy(sbt[:], p[:])
        kT.append(sbt)

    v_nat = kv_proj(wv, vbias_r if with_bias else None, "vn")
    v_m = v_nat[0]
    v_t = v_nat[1]
    v_tz = []
    for par in range(2):
        t = sb.tile([128, C], dt_att, tag="vtz", bufs=2)
        nc.vector.memset(t[:], 0)
        for pi in range(3):
            nc.sync.dma_start(t[32 * pi + 16 * par:32 * pi + 16 * par + 16, :], v_t[:])
        v_tz.append(t)
    wp = load_w("wpT", "wp")

    # ---- x tile loads ----
    def load_xg(g):
        ts_ = []
        for kc in range(8):
            t = sb.tile([128, GN * 128], dt_proj, tag="xg", bufs=16)
            nc.sync.dma_start(t[:], dram["xT"][kc * 128:(kc + 1) * 128,
                                               g * GN * 128:(g + 1) * GN * 128])
            ts_.append(t)
        return ts_

    xg_tiles = {0: load_xg(0), 1: load_xg(1)}
    qT_ref = {}     # (g, li) -> qpT sbuf tile
    dap_ref = {}    # (g, li) -> (dapm, dtl)
    rope_dma = {}   # (g, li) -> (cqt, nsqt, psqt)
    ou_ref = {}     # (g, chunk) -> ou sbuf tile

    # ================= pipeline stages =================
    def stage_A(g):
        if g + 2 < NG and (g + 2) not in xg_tiles:
            xg_tiles[g + 2] = load_xg(g + 2)
        xg = xg_tiles[g]
        for li in range(GN):
            lt = g * GN + li
            # rope tables for this l-tile
            cqt = sb.tile([128, 64], dt_att, tag="cqt", bufs=3)
            nc.sync.dma_start(cqt[:], dram["cq"][lt * 128:(lt + 1) * 128, :])
            nsqt = sb.tile([128, 32], dt_att, tag="nsqt", bufs=3)
            nc.sync.dma_start(nsqt[:], dram["nsq"][lt * 128:(lt + 1) * 128, :])
            psqt = sb.tile([128, 32], dt_att, tag="psqt", bufs=3)
            nc.sync.dma_start(psqt[:], dram["psq"][lt * 128:(lt + 1) * 128, :])
            # Q projection
            ps0 = ps.tile([128, 512], F32, tag="qp", bufs=2)
            ps1 = ps.tile([128, 512], F32, tag="qp", bufs=2)
            for kc in range(8):
                lhsT = xg[kc][:, li * 128:(li + 1) * 128]
                mm(ps0[:], lhsT, wq[kc][:, 0:512], (kc == 0), (kc == 7))
                mm(ps1[:], lhsT, wq[kc][:, 512:1024], (kc == 0), (kc == 7))
            q_sb = sb.tile([128, C], dt_att, tag="qsb", bufs=2)
            if with_bias:
                nc.vector.scalar_tensor_tensor(
                    q_sb[:, 0:512], ps0[:], 1.0, qbias_r[:, 0:512],
                    op0=OP.mult, op1=OP.add)
                nc.vector.scalar_tensor_tensor(
                    q_sb[:, 512:1024], ps1[:], 1.0, qbias_r[:, 512:1024],
                    op0=OP.mult, op1=OP.add)
            else:
                nc.scalar.copy(q_sb[:, 0:512], ps0[:])
                nc.scalar.copy(q_sb[:, 512:1024], ps1[:])
            # norm + rope (sq/hat/qb on GpSimd, rest DVE/ACT)
            sq = sb.tile([128, C], dt_att, tag="sq", bufs=2)
            (nc.gpsimd if USE_GP_SQ else nc.vector).tensor_mul(sq[:], q_sb[:], q_sb[:])
            s16 = sb.tile([128, H], F32, tag="s16", bufs=2)
            nc.vector.reduce_sum(s16[:], sq[:].rearrange("p (h d) -> p h d", d=D),
                                 axis=AX)
            rt = sb.tile([128, H], F32, tag="rt", bufs=2)
            nc.scalar.activation(rt[:], s16[:], AF.Sqrt, bias=eps[:])
            rq = sb.tile([128, H], F32, tag="rq", bufs=2)
            nc.vector.reciprocal(rq[:], rt[:])
            hat = sb.tile([128, C], dt_att, tag="hat", bufs=2)
            (nc.gpsimd if USE_GP_HAT else nc.vector).tensor_mul(
                hat[:].rearrange("p (h d) -> p h d", d=D),
                q_sb[:].rearrange("p (h d) -> p h d", d=D),
                rq[:].unsqueeze(2).to_broadcast((128, H, D)))
            qa = sb.tile([128, C], dt_att, tag="qa", bufs=2)
            nc.vector.tensor_mul(
                qa[:].rearrange("p (h d) -> p h d", d=D),
                hat[:].rearrange("p (h d) -> p h d", d=D),
                cqt[:].unsqueeze(1).to_broadcast((128, H, D)))
            qb = sb.tile([128, C], dt_att, tag="qb", bufs=2)
            hat4 = hat[:].rearrange("p (h j t) -> p h j t", j=32, t=2)
            qb4 = qb[:].rearrange("p (h j t) -> p h j t", j=32, t=2)
            qbeng = nc.gpsimd if USE_GP_QB else nc.vector
            qbeng.tensor_mul(
                qb4[:, :, :, 0:1].squeeze(3),
                hat4[:, :, :, 1:2].squeeze(3),
                nsqt[:].unsqueeze(1).to_broadcast((128, H, 32)))
            qbeng.tensor_mul(
                qb4[:, :, :, 1:2].squeeze(3),
                hat4[:, :, :, 0:1].squeeze(3),
                psqt[:].unsqueeze(1).to_broadcast((128, H, 32)))
            qp_sb = sb.tile([128, C], dt_att, tag="qp_sb", bufs=2)
            nc.vector.tensor_add(qp_sb[:], qa[:], qb[:])
            # transpose to [c-chunk blocks, l]
            psT = ps.tile([128, 1024], dt_att, tag="qpT", bufs=1)
            for ct in range(8):
                tr(psT[:, ct * 128:(ct + 1) * 128], qp_sb[:, ct * 128:(ct + 1) * 128],
                   ident[:])
            qT_sb = sb.tile([128, 1024], dt_att, tag="qT", bufs=6)
            nc.vector.tensor_copy(qT_sb[:], psT[:])
            qT_ref[(g, li)] = qT_sb

    def stage_B(g):
        for li in range(GN):
            lt = g * GN + li
            bias3_t = sb.tile([128, 3 * Lk], dt_att, tag="b3", bufs=3)
            nc.sync.dma_start(bias3_t[:], dram["bias3"][lt * 128:(lt + 1) * 128, :])
            s_all = sb.tile([128, H], F32, tag="sall", bufs=3)
            qT_sb = qT_ref[(g, li)]
            at_tiles = []
            for (h0, hn) in HG3:
                p = ps.tile([128, 512], F32, tag="lg", bufs=2)
                w = hn * Lk
                mm(p[:, 0:w], ident[:], bias3_t[:, 0:w], True, False)
                for j in range(hn):
                    h = h0 + j
                    t8 = h // 2
                    r0 = 64 * (h % 2)
                    mm(p[:, j * Lk:(j + 1) * Lk],
                       qT_sb[r0:r0 + 64, t8 * 128:(t8 + 1) * 128],
                       kT[t8][r0:r0 + 64, :],
                       False, (j == hn - 1))
                at = sb.tile([128, 3 * Lk], dt_att, tag="at", bufs=8)
                nc.scalar.activation(at[:, 0:w], p[:, 0:w], AF.Exp)
                nc.vector.reduce_sum(
                    s_all[:, h0:h0 + hn],
                    at[:, 0:w].rearrange("p (g k) -> p g k", k=Lk), axis=AX)
                at_tiles.append(at)
            rc = sb.tile([128, H], F32, tag="rcp", bufs=3)
            nc.vector.reciprocal(rc[:], s_all[:])
            dapm = sb.tile([128, 16 * 128], dt_att, tag="dm", bufs=4)
            dtl = sb.tile([128, 256], dt_att, tag="dtl", bufs=4)
            for h in range(H):
                at = at_tiles[h // 3]
                j = h % 3
                if USE_TS_DAP:
                    nc.vector.tensor_scalar_mul(
                        dapm[:, h * 128:(h + 1) * 128],
                        at[:, j * Lk:j * Lk + 128], rc[:, h:h + 1])
                    nc.vector.tensor_scalar_mul(
                        dtl[:, (h // 2) * 32 + (h % 2) * 16:(h // 2) * 32 + (h % 2) * 16 + 16],
                        at[:, j * Lk + 128:j * Lk + 144], rc[:, h:h + 1])
                else:
                    nc.vector.tensor_mul(
                        dapm[:, h * 128:(h + 1) * 128],
                        at[:, j * Lk:j * Lk + 128],
                        rc[:, h:h + 1].to_broadcast((128, 128)))
                    nc.vector.tensor_mul(
                        dtl[:, (h // 2) * 32 + (h % 2) * 16:(h // 2) * 32 + (h % 2) * 16 + 16],
                        at[:, j * Lk + 128:j * Lk + 144],
                        rc[:, h:h + 1].to_broadcast((128, 16)))
            dap_ref[(g, li)] = (dapm, dtl)

    def stage_C(g):
        daps = [dap_ref[(g, li)] for li in range(GN)]
        tailT = [None] * 3
        tail_need = {0: 0, 1: 1, 3: 2}  # chunk -> PRT batch to emit before it

        def emit_tail(tt):
            p0, pn = PRT[tt]
            p = ps.tile([32 * pn, GN * 128], dt_att, tag="tr2", bufs=2)
            for pi in range(pn):
                pr = p0 + pi
                for li in range(GN):
                    tr(p[32 * pi:32 * pi + 32, li * 128:(li + 1) * 128],
                       daps[li][1][:, pr * 32:(pr + 1) * 32], ident[:])
            t = sb.tile([32 * pn, GN * 128], dt_att, tag="tailT", bufs=3)
            nc.vector.tensor_copy(t[:], p[:])
            tailT[tt] = t

        for chunk in range(4):
            if chunk in tail_need:
                emit_tail(tail_need[chunk])
            # transpose 2 pairs (4 heads x 2 li) into one psum bank
            p = ps.tile([128, 1024], dt_att, tag="tr2", bufs=2)
            for pl in range(2):
                pr = 2 * chunk + pl
                for hh in range(2):
                    h = 2 * pr + hh
                    for li in range(GN):
                        tr(p[:, pl * 512 + hh * 256 + li * 128:
                             pl * 512 + hh * 256 + (li + 1) * 128],
                           daps[li][0][:, h * 128:(h + 1) * 128], ident[:])
            aTm = sb.tile([128, 1024], dt_att, tag="aTm", bufs=4)
            if chunk % 2 == 0:
                nc.scalar.copy(aTm[:], p[:])
            else:
                nc.vector.tensor_copy(aTm[:], p[:])
            # PV for the 2 pairs
            po = ps.tile([128, 512], F32, tag="po", bufs=1)
            for pl in range(2):
                pr = 2 * chunk + pl
                tt, pi = (pr // 3, pr % 3) if pr < 6 else (2, pr - 6)
                for hh in range(2):
                    h = 2 * pr + hh
                    reg = po[64 * hh:64 * hh + 64, pl * 256:(pl + 1) * 256]
                    mm(reg, v_m[:, h * D:(h + 1) * D],
                       aTm[:, pl * 512 + hh * 256:pl * 512 + (hh + 1) * 256],
                       True, False, skip_group_check=True)
                    mm(reg, v_tz[hh][32 * pi:32 * pi + 32, h * D:(h + 1) * D],
                       tailT[tt][32 * pi:32 * pi + 32, :],
                       False, True, skip_group_check=True,
                       tile_position=(32 * pi, 64 * hh))
            ou = sb.tile([128, 512], dt_proj, tag="ou", bufs=8)
            nc.scalar.copy(ou[:], po[:])
            ou_ref[(g, chunk)] = ou

    def stage_D(g):
        for li in range(GN):
            lt = g * GN + li
            ps0 = ps.tile([128, 512], F32, tag="qp", bufs=2)
            ps1 = ps.tile([128, 512], F32, tag="qp", bufs=2)
            for ct in range(8):
                lhsT = ou_ref[(g, ct // 2)][:, (ct % 2) * 256 + li * 128:
                                            (ct % 2) * 256 + (li + 1) * 128]
                mm(ps0[:], lhsT, wp[ct][:, 0:512], (ct == 0), (ct == 7))
                mm(ps1[:], lhsT, wp[ct][:, 512:1024], (ct == 0), (ct == 7))
            osb = sb.tile([128, C], F32, tag="osb", bufs=2)
            if with_bias:
                nc.vector.scalar_tensor_tensor(
                    osb[:, 0:512], ps0[:], 1.0, bproj_r[:, 0:512],
                    op0=OP.mult, op1=OP.add)
                nc.vector.scalar_tensor_tensor(
                    osb[:, 512:1024], ps1[:], 1.0, bproj_r[:, 512:1024],
                    op0=OP.mult, op1=OP.add)
            else:
                nc.scalar.copy(osb[:, 0:512], ps0[:])
                nc.scalar.copy(osb[:, 512:1024], ps1[:])
            nc.sync.dma_start(out_d[lt * 128:(lt + 1) * 128, :], osb[:])

    for s in range(NG + 2):
        if s < NG:
            stage_A(s)
        if 1 <= s <= NG:
            stage_B(s - 1)
        if s >= 2:
            stage_C(s - 2)
            stage_D(s - 2)
    ctx.close()


def run(inputs, dt_proj=BF16, dt_att=BF16, trace=False, nc=None):
    import ml_dtypes
    proj_np = ml_dtypes.bfloat16 if dt_proj == BF16 else np.float32
    with_bias = any(np.any(np.asarray(inputs[k])) for k in ("q_bias", "v_bias", "b_proj"))
    in_maps = host_prep(inputs, proj_np, with_bias)
    if nc is None:
        nc = build(dt_proj, dt_att, with_bias)
    res = run_bass_kernel_spmd(nc, in_maps, core_ids=list(range(8)), trace=trace)
    outs = np.stack([res.results[b]["out"][:L, :] for b in range(B)])
    return outs, res


if __name__ == "__main__":
    import time
    t0 = time.time()
    nc = build()
    print("BUILD OK", time.time() - t0, "s")


_NC_CACHE = {}


def kernel(**inputs):
    """Full unsharded inputs -> full output [8, 1704, 1024] float32.

    Data-parallel over batch: core b computes batch element b on NeuronCore b.
    """
    key_bias = bool(any(np.any(np.asarray(inputs[k]))
                        for k in ("q_bias", "v_bias", "b_proj")))
    key = (BF16, BF16, key_bias)
    if key not in _NC_CACHE:
        _NC_CACHE[key] = build(BF16, BF16, key_bias)
    out, _ = run(inputs, dt_proj=BF16, dt_att=BF16, trace=False, nc=_NC_CACHE[key])
    return out.astype(np.float32)
